# revision 20
# baseline (speedup 1.0000x reference)
"""CRF negative-log-likelihood kernel for Trainium2 (8 NeuronCores, SPMD).

Strategy
--------
Data-parallel over batch: core k owns sequences [64k, 64k+64).

The CRF forward (log-partition) recurrence runs in the exp domain:
    w_s = (E^T w_{s-1}) * Fhat_s          (per sequence, T=64-dim state)
with E = exp(transitions) and Fhat_s = exp(feats_s - c), c = log(64)+0.5 a
global constant keeping the state O(1); the host adds back (L-1)*c.

To halve the serial depth, each sequence is split at M in {127, 255}: the
forward recurrence runs s=0..M while the backward (beta) recurrence runs
s=L-1..M+1 — both at once, stacked on the 128 SBUF partitions (fwd tags on
partitions 0:64, bwd on 64:128) with stationary blockdiag(E, E^T).
256 macro-steps, each one 128x128->[128,64] bf16 matmul (PE) + one
elementwise multiply (DVE) against a precomputed schedule tensor Fsched.

Wall-clock of a warm call is dominated by shipping inputs over the axon
tunnel (~40 MB/s), so the schedule tensor is quantized to 4 bits and
nibble-packed on the host (8.4MB on the wire): the device splits nibbles
with two DVE shift ops and dequantizes inside the ACT exp via
exp(q*qs - c) (scale is a [128,1] AP so the per-call quantization scale
needs no recompile).  The quantization noise is uniform(-h, h) in log
space, which biases each live step by exactly ln(sinh(h)/h); the host
subtracts (L-1)*ln(sinh(h)/h) per sequence, leaving only a random-walk
residual that averages out across the batch.  The backward-chain boot
injections (one delta at STOP per sequence) are rank-1: a [1,64]
E[:,STOP] row times a [1, 129*n] one-hot boot-mask row, combined on
device by a K=1 outer-product matmul — 132KB on the wire instead of an
8MB one-hot tensor.  The jitted shard_map callable is cached across
calls, the tiny per-core outputs are fetched with parallel per-shard
RPCs, and the gold-score computation overlaps the device round-trip.
"""
import sys

for _p in ("/opt/trn_rl_repo",):
    if _p not in sys.path:
        sys.path.insert(0, _p)

import numpy as np
import ml_dtypes

BF16 = ml_dtypes.bfloat16

B, S, T = 512, 512, 64
N_CORES = 8
SEQ_PER_CORE = B // N_CORES          # 64
NSTEP = 256
START, STOP = T - 2, T - 1
C_NORM = float(np.log(64.0) + 0.5)
NBOOT_BWD = 129                      # bwd boot window: steps 1..129

PACK4 = True                         # 4-bit nibble-packed schedule tensor
QLEVELS = 7 if PACK4 else 127        # quant grid: q in [-QLEVELS, QLEVELS]
N_CHUNKS = 2                         # pipeline: prep chunk k+1 under chunk
                                     # k's wire transfer; round-trips overlap

_PROG_CACHE = {}


def _build_program(n):
    import concourse.bacc as bacc
    import concourse.mybir as mybir
    from concourse.tile import TileContext

    f32 = mybir.dt.float32
    bf16 = mybir.dt.bfloat16
    i8 = mybir.dt.int8
    fcols = 64 if PACK4 else 128     # feats_sched byte-columns per row

    nc = bacc.Bacc()
    feats_sched = nc.declare_dram_parameter(
        "feats_sched", [n * NSTEP, fcols], i8, isOutput=False)
    w_in = nc.declare_dram_parameter("w_blocks", [128, 128], bf16, isOutput=False)
    inj_a = nc.declare_dram_parameter("inj_a", [T, n], bf16, isOutput=False)
    inj_b = nc.declare_dram_parameter("inj_b", [T, n], bf16, isOutput=False)
    est_in = nc.declare_dram_parameter("est", [1, T], bf16, isOutput=False)
    mrow_in = nc.declare_dram_parameter(
        "mrow", [1, NBOOT_BWD * n], bf16, isOutput=False)
    qs_in = nc.declare_dram_parameter("qscale", [128, 4], f32, isOutput=False)
    out_s = nc.declare_dram_parameter("out_s", [1, n], f32, isOutput=True)

    EXP = mybir.ActivationFunctionType.Exp
    AND = mybir.AluOpType.bitwise_and
    XOR = mybir.AluOpType.bitwise_xor
    SUB = mybir.AluOpType.subtract

    with TileContext(nc) as tc:
        with (
            tc.tile_pool(name="persist", bufs=1) as pp,
            tc.tile_pool(name="stage", bufs=3) as sp,
            tc.tile_pool(name="dram", bufs=1, space="DRAM") as dp,
            tc.tile_pool(name="psum", bufs=1, space="PSUM") as psp,
        ):
            # [tag-dims, window, slot, col-in-window]: each window's
            # transpose output is contiguous (the DMA xbar ignores
            # strided 3D out APs and writes contiguously)
            Fs = pp.tile([128, NSTEP // 16, n, 16], bf16)
            Z = pp.tile([128, n], bf16)
            W = pp.tile([128, 128], bf16)           # blockdiag(E, E^T)
            IA = pp.tile([T, n], bf16)
            IB = pp.tile([T, n], bf16)
            EST = pp.tile([1, T], bf16)
            MROW = pp.tile([1, NBOOT_BWD * n], bf16)
            QSC = pp.tile([128, 4], f32)
            ONES = pp.tile([T, 1], f32)
            PROD = pp.tile([T, n], f32)
            OUT = pp.tile([1, n], f32)

            nc.sync.dma_start(W[:], w_in[:])
            nc.sync.dma_start(IA[:], inj_a[:])
            nc.sync.dma_start(IB[:], inj_b[:])
            nc.sync.dma_start(EST[:], est_in[:])
            nc.sync.dma_start(MROW[:], mrow_in[:])
            nc.sync.dma_start(QSC[:], qs_in[:])
            nc.vector.memset(Z[:], 0.0)
            nc.vector.memset(ONES[:], 1.0)

            # ---- precompute Fsched: exp(q*qs - c) transposed ----
            # feats_sched rows are window-major: row = w*16n + v*16 + c_i
            # (slot v, step-col 16w + c_i); byte-cols cover 128 tag-dims
            # (fwd seq tags 0:64 | bwd seq tags 64:128), nibble-packed in
            # pairs when PACK4 (byte b holds tags 2b, 2b+1).  Each 16-step
            # window: contiguous int8 load -> nibble split (DVE) -> ACT exp
            # (dequant fused via scale/bias APs) -> bf16 scratch -> one big
            # DMA-xbar transpose into Fsched (consumers wait on exactly one
            # DMA each).
            rpw = 16 * n                 # rows per window
            g = rpw // 128               # dram rows folded into the free dim
            scratch = dp.tile([n * NSTEP, 128], bf16)
            fsv = feats_sched[:].rearrange(
                "(w p g) t -> w p (g t)", p=128, g=g)
            scv = scratch[:].rearrange("(w p g) t -> w p (g t)", p=128, g=g)
            for w in range(NSTEP // 16):
                stg = sp.tile([128, g * fcols], i8, tag="stg_in")
                nc.sync.dma_start(stg[:], fsv[w])
                # dedicated mid tile per window: the exp never carries a
                # write-after-read wait (ISA sync-slot budget on ACT is tiny)
                mid = pp.tile([128, rpw], bf16, tag=f"mid{w}")
                if PACK4:
                    # byte b = 16*q_hi + (q_lo & 15); t1 = b & 15, so
                    # b - t1 = 16*q_hi exactly and (t1 ^ 8) = q_lo + 8.
                    # Byte flat-index i unpacks to elements 2i (lo) and
                    # 2i+1 (hi); dequant folds into the ACT scale/bias:
                    # lo: exp(t1x*qs - (c+8qs)), hi: exp(d*(qs/16) - c).
                    t1 = sp.tile([128, rpw // 2], i8, tag="t1")
                    lo = sp.tile([128, rpw // 2], i8, tag="lo")
                    dd = sp.tile([128, rpw // 2], i8, tag="dd")
                    nc.vector.tensor_scalar(t1[:], stg[:], 15, None, AND)
                    nc.vector.tensor_scalar(lo[:], t1[:], 8, None, XOR)
                    nc.vector.tensor_tensor(dd[:], stg[:], t1[:], SUB)
                    mv = mid[:].rearrange("p (i two) -> p two i", two=2)
                    nc.scalar.activation(mv[:, 0], lo[:], EXP,
                                         bias=QSC[:, 1:2], scale=QSC[:, 0:1])
                    nc.scalar.activation(mv[:, 1], dd[:], EXP,
                                         bias=QSC[:, 3:4], scale=QSC[:, 2:3])
                else:
                    nc.scalar.activation(mid[:], stg[:], EXP,
                                         bias=QSC[:, 3:4], scale=QSC[:, 0:1])
                nc.sync.dma_start(scv[w], mid[:])
                nc.sync.dma_start_transpose(
                    Fs[:, w], scratch[w * rpw:(w + 1) * rpw, :])

            # ---- the 256-step meet-in-the-middle scan ----
            sink = pp.tile([1, 16], bf16)
            for i in range(1, NSTEP + 1):
                if (i - 1) % 16 == 0:
                    # absorb the Fsched-transpose DMA wait on a cheap DVE op
                    nc.vector.tensor_copy(
                        sink[:], Fs[0:1, (i - 1) // 16, 0:1, :])
                has_fa = i == 2
                has_fb = i == 130
                has_bw = i <= NBOOT_BWD
                n_mm = 1 + has_fa + has_fb + has_bw
                ps = psp.tile([128, n], mybir.dt.float32, tag="scanps")
                k = 1
                nc.tensor.matmul(ps[:], W[:], Z[:], start=True,
                                 stop=(k == n_mm))
                if has_fa:
                    k += 1
                    nc.tensor.matmul(ps[0:64, :], W[0:64, 0:64], IA[:],
                                     start=False, stop=(k == n_mm))
                if has_fb:
                    k += 1
                    nc.tensor.matmul(ps[0:64, :], W[0:64, 0:64], IB[:],
                                     start=False, stop=(k == n_mm))
                if has_bw:
                    k += 1
                    # rank-1 boot: outer(E[:,STOP], boot-mask row, step i)
                    nc.tensor.matmul(ps[64:128, :], EST[:],
                                     MROW[0:1, (i - 1) * n:i * n],
                                     start=False, stop=(k == n_mm))
                nc.vector.tensor_mul(
                    Z[:], ps[:], Fs[:, (i - 1) // 16, :, (i - 1) % 16])

            # ---- final combine: S = sum_t Zfwd * (E @ Zbwd) ----
            psD = psp.tile([T, n], mybir.dt.float32, tag="scanps")
            nc.tensor.matmul(psD[:], W[64:128, 64:128], Z[64:128, :],
                             start=True, stop=True)
            nc.vector.tensor_mul(PROD[:], psD[:], Z[0:64, :])
            psS = psp.tile([1, n], mybir.dt.float32, tag="scanps")
            nc.tensor.matmul(psS[:], ONES[:], PROD[:], start=True, stop=True)
            nc.vector.tensor_copy(OUT[:], psS[:])
            nc.sync.dma_start(out_s[:], OUT[:])

    nc.finalize()
    return nc


def _get_program(n):
    key = ("nc", n)
    if key not in _PROG_CACHE:
        _PROG_CACHE[key] = _build_program(n)
    return _PROG_CACHE[key]


def _get_runner(n):
    """Cached jitted shard_map callable over the 8 cores.

    Replicates concourse.bass2jax.run_bass_via_pjrt but holds onto the
    jitted function: the library rebuilds jax.jit(shard_map(...)) on every
    call, which forces a full retrace; caching it cuts several hundred ms
    off each warm call.  Returns an async launcher: run(in_map) dispatches
    and returns a fetch() closure so host work can overlap the device
    round-trip; outputs are fetched shard-parallel (8 small RPCs in
    flight beats one serialized gather by ~80ms on this tunnel).
    """
    key = ("runner", n)
    if key in _PROG_CACHE:
        return _PROG_CACHE[key]

    import jax
    from jax.sharding import Mesh, PartitionSpec
    from jax.experimental.shard_map import shard_map
    from concurrent.futures import ThreadPoolExecutor
    import concourse.mybir as mybir
    from concourse.bass2jax import (
        _bass_exec_p, install_neuronx_cc_hook, partition_id_tensor)

    nc = _get_program(n)
    install_neuronx_cc_hook()

    partition_name = (
        nc.partition_id_tensor.name if nc.partition_id_tensor else None)
    in_names: list[str] = []
    out_names: list[str] = []
    out_avals = []
    for alloc in nc.m.functions[0].allocations:
        if not isinstance(alloc, mybir.MemoryLocationSet):
            continue
        name = alloc.memorylocations[0].name
        if alloc.kind == "ExternalInput":
            if name != partition_name:
                in_names.append(name)
        elif alloc.kind == "ExternalOutput":
            shape = tuple(alloc.tensor_shape)
            dtype = mybir.dt.np(alloc.dtype)
            out_names.append(name)
            out_avals.append(jax.core.ShapedArray(shape, dtype))
    n_params = len(in_names)
    n_outs = len(out_avals)
    dbg_name = nc.dbg_addr.name if nc.dbg_addr is not None else None
    if dbg_name is not None and dbg_name in in_names:
        in_names.remove(dbg_name)
        n_params -= 1
    in_names_full = (list(in_names)
                     + ([dbg_name] if dbg_name else [])
                     + out_names
                     + ([partition_name] if partition_name else []))
    n_extra = 1 if dbg_name else 0
    donate = tuple(range(n_params + n_extra, n_params + n_extra + n_outs))

    def _body(*args):
        operands = list(args)
        if partition_name is not None:
            operands.append(partition_id_tensor())
        outs = _bass_exec_p.bind(
            *operands,
            out_avals=tuple(out_avals),
            in_names=tuple(in_names_full),
            out_names=tuple(out_names),
            lowering_input_output_aliases=(),
            sim_require_finite=True,
            sim_require_nnan=True,
            nc=nc,
        )
        return tuple(outs)

    devices = jax.devices()[:N_CORES]
    mesh = Mesh(np.asarray(devices), ("core",))
    n_all = n_params + n_extra + n_outs
    sharded = jax.jit(
        shard_map(_body, mesh=mesh,
                  in_specs=(PartitionSpec("core"),) * n_all,
                  out_specs=(PartitionSpec("core"),) * n_outs,
                  check_rep=False),
        donate_argnums=donate, keep_unused=True)
    pool = ThreadPoolExecutor(N_CORES)

    def run(in_map):
        args = [in_map[nm] for nm in in_names]
        if dbg_name:
            args.append(np.zeros((N_CORES, 2), np.uint32))
        zeros = [np.zeros((N_CORES * a.shape[0], *a.shape[1:]), a.dtype)
                 for a in out_avals]
        outs = sharded(*args, *zeros)   # async dispatch

        def fetch():
            res = {}
            for i, nm in enumerate(out_names):
                parts = list(pool.map(
                    lambda s: np.asarray(s.data), outs[i].addressable_shards))
                res[nm] = np.stack(parts).reshape(
                    N_CORES, *out_avals[i].shape)
            return res
        return fetch

    _PROG_CACHE[key] = run
    return run


def _host_prep_global(feats, lengths, transitions):
    """Vectorized build of the globally-concatenated device inputs.

    Global layout = per-core arrays concatenated on axis 0 (core-major);
    feats holds the n_chunk*8 sequences of this chunk, core-major.
    Returns (in_map, delta) where delta is the per-live-step quantization
    bias ln(sinh(h)/h) the host must subtract (L-1) times per sequence.
    """
    Bc = feats.shape[0]
    n = Bc // N_CORES
    L = lengths.astype(np.int64)                  # (Bc,)
    M = np.where(L <= 383, 127, 255)              # (B,) split point
    t64 = transitions.astype(np.float64)
    E = np.exp(t64)
    Wb = np.zeros((128, 128), np.float32)
    Wb[0:64, 0:64] = E
    Wb[64:128, 64:128] = E.T
    w_blocks_g = np.tile(Wb.astype(BF16), (N_CORES, 1))

    # quantize feats to q in [-QLEVELS, QLEVELS]; scale ships as an AP.
    # absmax from a subsample (the explicit clip bounds the rare overshoot)
    # and a cache-blocked quantize+pack: one streaming read of feats, one
    # int8 write, instead of five full-memory sweeps.
    absmax = float(np.abs(feats.reshape(-1)[::499]).max()) * 1.04
    qs = max(absmax, 1e-30) / QLEVELS
    inv = np.float32(1.0 / qs)
    hc = T // 2 if PACK4 else T
    fq = np.empty((feats.shape[0], S, hc), np.int8)
    blk = 16
    tmp = np.empty((blk, S, T), np.float32)
    for b0 in range(0, feats.shape[0], blk):
        nb = min(blk, feats.shape[0] - b0)
        t = tmp[:nb]
        np.multiply(feats[b0:b0 + nb], inv, out=t)
        np.rint(t, out=t)
        np.clip(t, -QLEVELS, QLEVELS, out=t)
        q = t.astype(np.int8)
        if PACK4:
            # byte b holds tag pair (2b, 2b+1): lo nibble even, hi odd
            fq[b0:b0 + nb] = (q[:, :, 1::2] << 4) | (q[:, :, 0::2] & 0x0F)
        else:
            fq[b0:b0 + nb] = q
    h = 0.5 * qs
    delta = float(np.log(np.sinh(h) / h)) if h > 1e-12 else 0.0

    # schedule columns: step i (1..256) consumes column j = i-1.
    # fwd chain of seq v: col j holds feats[s] with s = j - (255-M), live
    # for 1 <= s <= M (tags 0:64).  bwd chain: s = 256 + M - j, live for
    # M+1 <= s <= L-1 (tags 64:128).  Dead slots are 0 -> exp(-c).
    # The shift (255-M) is only ever 0 or 128, so the gathers are two
    # boolean-row slice copies (sequential memcpy beats take_along_axis
    # ~4x); the validity-mask multiply zeroes the dead slots after.
    j = np.arange(NSTEP)[None, :]                 # (1,256)
    sf = j - (255 - M)[:, None]                   # (Bc,256)
    vf = (sf >= 1) & (sf <= M[:, None])
    sb = 256 + M[:, None] - j                     # (Bc,256)
    vb = (sb > M[:, None]) & (sb <= (L - 1)[:, None])
    m255 = M == 255
    m127 = ~m255
    sched = np.empty((Bc, NSTEP, 2 * hc), np.int8)
    gf = sched[:, :, 0:hc]
    gb = sched[:, :, hc:2 * hc]
    fqr = fq[:, ::-1]                             # fqr[v, j] = fq[v, 511-j]
    gf[m255] = fq[m255, 0:NSTEP]
    gf[m127, 128:NSTEP] = fq[m127, 0:128]
    gb[m255] = fqr[m255, 0:NSTEP]
    gb[m127] = fqr[m127, 128:128 + NSTEP]
    gf *= vf[:, :, None]
    gb *= vb[:, :, None]
    # window-major per-core rows: row = w*1024 + v*16 + c_i
    fs_g = (sched.reshape(N_CORES, n, NSTEP // 16, 16, 2 * hc)
                 .transpose(0, 2, 1, 3, 4)
                 .reshape(N_CORES * n * NSTEP, 2 * hc))

    # fwd boots: w0 = exp(feats[:,0] + trans[START]); injected at step 2
    # (M=255 chains, IA) or step 130 (M=127 chains, IB)
    w0 = np.exp(feats[:, 0, :].astype(np.float64) + t64[START][None, :])
    ia = np.where((M == 255)[:, None], w0, 0.0)
    ib = np.where((M == 127)[:, None], w0, 0.0)
    ia_g = ia.reshape(N_CORES, n, T).transpose(0, 2, 1).reshape(N_CORES * T, n)
    ib_g = ib.reshape(N_CORES, n, T).transpose(0, 2, 1).reshape(N_CORES * T, n)

    # bwd boots: delta at STOP injected at step i0b = 258 + M - L, encoded
    # rank-1 as E[:,STOP] (est row) x one-hot boot-mask row
    est_g = np.tile(E[:, STOP].astype(BF16)[None, :], (N_CORES, 1))
    i0b = 258 + M - L                             # (Bc,) in [1, 129]
    mrow_g = np.zeros((N_CORES, NBOOT_BWD * n), BF16)
    bidx = np.arange(Bc)
    mrow_g[bidx // n, (i0b - 1) * n + (bidx % n)] = 1.0

    # ACT scale/bias table: [qs, -(c+8qs), qs/16, -c] per partition
    qs_g = np.tile(
        np.array([[qs, -(C_NORM + 8.0 * qs), qs / 16.0, -C_NORM]],
                 np.float32),
        (N_CORES * 128, 1))

    return {
        "feats_sched": fs_g,
        "w_blocks": w_blocks_g,
        "inj_a": ia_g.astype(BF16),
        "inj_b": ib_g.astype(BF16),
        "est": est_g,
        "mrow": mrow_g,
        "qscale": qs_g,
    }, delta


def _gold_score(feats, mask, tags, transitions):
    t64 = transitions.astype(np.float64)
    prev = np.concatenate(
        [np.full((B, 1), START, dtype=tags.dtype), tags[:, :-1]], axis=1)
    emit = np.take_along_axis(
        feats, tags[:, :, None].astype(np.int64), axis=2)[:, :, 0]
    tg = emit.astype(np.float64) + t64[prev, tags]
    gold = np.where(mask, tg, 0.0).sum()
    lengths = mask.sum(axis=1).astype(np.int64)
    end_ids = np.take_along_axis(tags, (lengths - 1)[:, None].astype(tags.dtype),
                                 axis=1)[:, 0]
    return gold + t64[end_ids, STOP].sum()


def kernel(feats, mask, tags, transitions, _trace=False):
    feats = np.asarray(feats, dtype=np.float32)
    mask = np.asarray(mask)
    tags = np.asarray(tags)
    transitions = np.asarray(transitions, dtype=np.float32)
    lengths = mask.astype(np.int64).sum(axis=1)

    # chunk c takes slots [cW, cW+W) of every core (core-major order)
    W = SEQ_PER_CORE // N_CHUNKS
    n = W
    chunk_idx = [
        (np.arange(N_CORES)[:, None] * SEQ_PER_CORE
         + c * W + np.arange(W)[None, :]).ravel()
        for c in range(N_CHUNKS)]

    if _trace:
        from concourse.bass_utils import run_bass_kernel_spmd
        nc = _get_program(n)
        forward = 0.0
        for c in range(N_CHUNKS):
            bi = chunk_idx[c]
            glob_in, delta = _host_prep_global(
                feats[bi], lengths[bi], transitions)
            in_maps = []
            for k in range(N_CORES):
                m = {}
                for nm, g in glob_in.items():
                    rows = g.shape[0] // N_CORES
                    m[nm] = g[k * rows:(k + 1) * rows]
                in_maps.append(m)
            res = run_bass_kernel_spmd(
                nc, in_maps, core_ids=list(range(N_CORES)), trace=True)
            _PROG_CACHE["last_result"] = res
            svec = np.concatenate(
                [res.results[k]["out_s"][0].astype(np.float64)
                 for k in range(N_CORES)])
            l1 = lengths[bi].astype(np.float64) - 1.0
            forward += (np.log(svec) + l1 * (C_NORM - delta)).sum()
        gold = _gold_score(feats, mask, tags, transitions)
        return np.float32(forward - gold)

    runner = _get_runner(n)
    fetches = []
    deltas = []
    for c in range(N_CHUNKS):
        bi = chunk_idx[c]
        glob_in, delta = _host_prep_global(
            feats[bi], lengths[bi], transitions)
        fetches.append(runner(glob_in))          # async dispatch
        deltas.append(delta)
    gold = _gold_score(feats, mask, tags, transitions)
    forward = 0.0
    for c in range(N_CHUNKS):
        svec = fetches[c]()["out_s"].reshape(-1).astype(np.float64)
        l1 = lengths[chunk_idx[c]].astype(np.float64) - 1.0
        forward += (np.log(svec) + l1 * (C_NORM - deltas[c])).sum()
    return np.float32(forward - gold)


# revision 31
# speedup vs baseline: 1.4586x; 1.4586x over previous
"""CRF negative-log-likelihood kernel for Trainium2 (8 NeuronCores, SPMD).

Strategy
--------
Data-parallel over batch: core k owns sequences [64k, 64k+64).

The CRF forward (log-partition) recurrence runs in the exp domain:
    w_s = (E^T w_{s-1}) * Fhat_s          (per sequence, T=64-dim state)
with E = exp(transitions) and Fhat_s = exp(feats_s - c), c = log(64)+0.5 a
global constant keeping the state O(1); the host adds back (L-1)*c.

To halve the serial depth, each sequence is split at M in {127, 255}: the
forward recurrence runs s=0..M while the backward (beta) recurrence runs
s=L-1..M+1 — both at once, stacked on the 128 SBUF partitions (fwd tags on
partitions 0:64, bwd on 64:128) with stationary blockdiag(E, E^T).
256 macro-steps, each one 128x128->[128,64] bf16 matmul (PE) + one
elementwise multiply (DVE) against a precomputed schedule tensor Fsched.

Wall-clock of a warm call is dominated by shipping inputs over the axon
tunnel (~40 MB/s), so the schedule tensor is quantized to 4 bits and
nibble-packed on the host (8.4MB on the wire): the device splits nibbles
with two DVE shift ops and dequantizes inside the ACT exp via
exp(q*qs - c) (scale is a [128,1] AP so the per-call quantization scale
needs no recompile).  The quantization noise is uniform(-h, h) in log
space, which biases each live step by exactly ln(sinh(h)/h); the host
subtracts (L-1)*ln(sinh(h)/h) per sequence, leaving only a random-walk
residual that averages out across the batch.  The backward-chain boot
injections (one delta at STOP per sequence) are rank-1: a [1,64]
E[:,STOP] row times a [1, 129*n] one-hot boot-mask row, combined on
device by a K=1 outer-product matmul — 132KB on the wire instead of an
8MB one-hot tensor.  The jitted shard_map callable is cached across
calls, the tiny per-core outputs are fetched with parallel per-shard
RPCs, and the gold-score computation overlaps the device round-trip.
"""
import sys

for _p in ("/opt/trn_rl_repo",):
    if _p not in sys.path:
        sys.path.insert(0, _p)

import numpy as np
import ml_dtypes

BF16 = ml_dtypes.bfloat16

B, S, T = 512, 512, 64
N_CORES = 8
SEQ_PER_CORE = B // N_CORES          # 64
NSTEP = 256
START, STOP = T - 2, T - 1
C_NORM = float(np.log(64.0) + 0.5)
NBOOT_BWD = 129                      # bwd boot window: steps 1..129

PACK4 = True                         # 4-bit nibble-packed schedule tensor
QLEVELS = 7 if PACK4 else 127        # quant grid: q in [-QLEVELS, QLEVELS]
N_CHUNKS = 1                         # >1 pipelines chunk prep under the wire
                                     # transfer, but each extra runner call
                                     # costs ~80ms serial RTT on this tunnel
                                     # (calls do not overlap), so 1 is best

_PROG_CACHE = {}


def _build_program(n):
    import concourse.bacc as bacc
    import concourse.mybir as mybir
    from concourse.tile import TileContext

    f32 = mybir.dt.float32
    bf16 = mybir.dt.bfloat16
    i8 = mybir.dt.int8
    fcols = 64 if PACK4 else 128     # feats_sched byte-columns per row

    nc = bacc.Bacc()
    feats_sched = nc.declare_dram_parameter(
        "feats_sched", [n * NSTEP, fcols], i8, isOutput=False)
    w_in = nc.declare_dram_parameter("w_blocks", [128, 128], bf16, isOutput=False)
    inj_a = nc.declare_dram_parameter("inj_a", [T, n], bf16, isOutput=False)
    inj_b = nc.declare_dram_parameter("inj_b", [T, n], bf16, isOutput=False)
    est_in = nc.declare_dram_parameter("est", [1, T], bf16, isOutput=False)
    mrow_in = nc.declare_dram_parameter(
        "mrow", [1, NBOOT_BWD * n], bf16, isOutput=False)
    qs_in = nc.declare_dram_parameter("qscale", [128, 4], f32, isOutput=False)
    out_s = nc.declare_dram_parameter("out_s", [1, n], f32, isOutput=True)

    EXP = mybir.ActivationFunctionType.Exp
    AND = mybir.AluOpType.bitwise_and
    XOR = mybir.AluOpType.bitwise_xor
    SUB = mybir.AluOpType.subtract

    with TileContext(nc) as tc:
        with (
            tc.tile_pool(name="persist", bufs=1) as pp,
            tc.tile_pool(name="stage", bufs=3) as sp,
            tc.tile_pool(name="dram", bufs=1, space="DRAM") as dp,
            tc.tile_pool(name="psum", bufs=1, space="PSUM") as psp,
        ):
            # [tag-dims, window, slot, col-in-window]: each window's
            # transpose output is contiguous (the DMA xbar ignores
            # strided 3D out APs and writes contiguously)
            Fs = pp.tile([128, NSTEP // 16, n, 16], bf16)
            Z = pp.tile([128, n], bf16)
            W = pp.tile([128, 128], bf16)           # blockdiag(E, E^T)
            IA = pp.tile([T, n], bf16)
            IB = pp.tile([T, n], bf16)
            EST = pp.tile([1, T], bf16)
            MROW = pp.tile([1, NBOOT_BWD * n], bf16)
            QSC = pp.tile([128, 4], f32)
            ONES = pp.tile([T, 1], f32)
            PROD = pp.tile([T, n], f32)
            OUT = pp.tile([1, n], f32)

            nc.sync.dma_start(W[:], w_in[:])
            nc.sync.dma_start(IA[:], inj_a[:])
            nc.sync.dma_start(IB[:], inj_b[:])
            nc.sync.dma_start(EST[:], est_in[:])
            nc.sync.dma_start(MROW[:], mrow_in[:])
            nc.sync.dma_start(QSC[:], qs_in[:])
            nc.vector.memset(Z[:], 0.0)
            nc.vector.memset(ONES[:], 1.0)

            # ---- precompute Fsched: exp(q*qs - c) transposed ----
            # feats_sched rows are window-major: row = w*16n + v*16 + c_i
            # (slot v, step-col 16w + c_i); byte-cols cover 128 tag-dims
            # (fwd seq tags 0:64 | bwd seq tags 64:128), nibble-packed in
            # pairs when PACK4 (byte b holds tags 2b, 2b+1).  Each 16-step
            # window: contiguous int8 load -> nibble split (DVE) -> ACT exp
            # (dequant fused via scale/bias APs) -> bf16 scratch -> one big
            # DMA-xbar transpose into Fsched (consumers wait on exactly one
            # DMA each).
            rpw = 16 * n                 # rows per window
            g = rpw // 128               # dram rows folded into the free dim
            scratch = dp.tile([n * NSTEP, 128], bf16)
            fsv = feats_sched[:].rearrange(
                "(w p g) t -> w p (g t)", p=128, g=g)
            scv = scratch[:].rearrange("(w p g) t -> w p (g t)", p=128, g=g)
            for w in range(NSTEP // 16):
                stg = sp.tile([128, g * fcols], i8, tag="stg_in")
                nc.sync.dma_start(stg[:], fsv[w])
                # dedicated mid tile per window: the exp never carries a
                # write-after-read wait (ISA sync-slot budget on ACT is tiny)
                mid = pp.tile([128, rpw], bf16, tag=f"mid{w}")
                if PACK4:
                    # byte b = 16*q_hi + (q_lo & 15); t1 = b & 15, so
                    # b - t1 = 16*q_hi exactly and (t1 ^ 8) = q_lo + 8.
                    # Byte flat-index i unpacks to elements 2i (lo) and
                    # 2i+1 (hi); dequant folds into the ACT scale/bias:
                    # lo: exp(t1x*qs - (c+8qs)), hi: exp(d*(qs/16) - c).
                    t1 = sp.tile([128, rpw // 2], i8, tag="t1")
                    lo = sp.tile([128, rpw // 2], i8, tag="lo")
                    dd = sp.tile([128, rpw // 2], i8, tag="dd")
                    nc.vector.tensor_scalar(t1[:], stg[:], 15, None, AND)
                    nc.vector.tensor_scalar(lo[:], t1[:], 8, None, XOR)
                    nc.vector.tensor_tensor(dd[:], stg[:], t1[:], SUB)
                    mv = mid[:].rearrange("p (i two) -> p two i", two=2)
                    nc.scalar.activation(mv[:, 0], lo[:], EXP,
                                         bias=QSC[:, 1:2], scale=QSC[:, 0:1])
                    nc.scalar.activation(mv[:, 1], dd[:], EXP,
                                         bias=QSC[:, 3:4], scale=QSC[:, 2:3])
                else:
                    nc.scalar.activation(mid[:], stg[:], EXP,
                                         bias=QSC[:, 3:4], scale=QSC[:, 0:1])
                nc.sync.dma_start(scv[w], mid[:])
                nc.sync.dma_start_transpose(
                    Fs[:, w], scratch[w * rpw:(w + 1) * rpw, :])

            # ---- the 256-step meet-in-the-middle scan ----
            sink = pp.tile([1, 16], bf16)
            for i in range(1, NSTEP + 1):
                if (i - 1) % 16 == 0:
                    # absorb the Fsched-transpose DMA wait on a cheap DVE op
                    nc.vector.tensor_copy(
                        sink[:], Fs[0:1, (i - 1) // 16, 0:1, :])
                has_fa = i == 2
                has_fb = i == 130
                has_bw = i <= NBOOT_BWD
                n_mm = 1 + has_fa + has_fb + has_bw
                ps = psp.tile([128, n], mybir.dt.float32, tag="scanps")
                k = 1
                nc.tensor.matmul(ps[:], W[:], Z[:], start=True,
                                 stop=(k == n_mm))
                if has_fa:
                    k += 1
                    nc.tensor.matmul(ps[0:64, :], W[0:64, 0:64], IA[:],
                                     start=False, stop=(k == n_mm))
                if has_fb:
                    k += 1
                    nc.tensor.matmul(ps[0:64, :], W[0:64, 0:64], IB[:],
                                     start=False, stop=(k == n_mm))
                if has_bw:
                    k += 1
                    # rank-1 boot: outer(E[:,STOP], boot-mask row, step i)
                    nc.tensor.matmul(ps[64:128, :], EST[:],
                                     MROW[0:1, (i - 1) * n:i * n],
                                     start=False, stop=(k == n_mm))
                nc.vector.tensor_mul(
                    Z[:], ps[:], Fs[:, (i - 1) // 16, :, (i - 1) % 16])

            # ---- final combine: S = sum_t Zfwd * (E @ Zbwd) ----
            psD = psp.tile([T, n], mybir.dt.float32, tag="scanps")
            nc.tensor.matmul(psD[:], W[64:128, 64:128], Z[64:128, :],
                             start=True, stop=True)
            nc.vector.tensor_mul(PROD[:], psD[:], Z[0:64, :])
            psS = psp.tile([1, n], mybir.dt.float32, tag="scanps")
            nc.tensor.matmul(psS[:], ONES[:], PROD[:], start=True, stop=True)
            nc.vector.tensor_copy(OUT[:], psS[:])
            nc.sync.dma_start(out_s[:], OUT[:])

    nc.finalize()
    return nc


def _get_program(n):
    key = ("nc", n)
    if key not in _PROG_CACHE:
        _PROG_CACHE[key] = _build_program(n)
    return _PROG_CACHE[key]


def _get_runner(n):
    """Cached jitted shard_map callable over the 8 cores.

    Replicates concourse.bass2jax.run_bass_via_pjrt but holds onto the
    jitted function: the library rebuilds jax.jit(shard_map(...)) on every
    call, which forces a full retrace; caching it cuts several hundred ms
    off each warm call.  Returns an async launcher: run(in_map) dispatches
    and returns a fetch() closure so host work can overlap the device
    round-trip; outputs are fetched shard-parallel (8 small RPCs in
    flight beats one serialized gather by ~80ms on this tunnel).
    """
    key = ("runner", n)
    if key in _PROG_CACHE:
        return _PROG_CACHE[key]

    import jax
    from jax.sharding import Mesh, PartitionSpec
    from jax.experimental.shard_map import shard_map
    from concurrent.futures import ThreadPoolExecutor
    import concourse.mybir as mybir
    from concourse.bass2jax import (
        _bass_exec_p, install_neuronx_cc_hook, partition_id_tensor)

    nc = _get_program(n)
    install_neuronx_cc_hook()

    partition_name = (
        nc.partition_id_tensor.name if nc.partition_id_tensor else None)
    in_names: list[str] = []
    out_names: list[str] = []
    out_avals = []
    for alloc in nc.m.functions[0].allocations:
        if not isinstance(alloc, mybir.MemoryLocationSet):
            continue
        name = alloc.memorylocations[0].name
        if alloc.kind == "ExternalInput":
            if name != partition_name:
                in_names.append(name)
        elif alloc.kind == "ExternalOutput":
            shape = tuple(alloc.tensor_shape)
            dtype = mybir.dt.np(alloc.dtype)
            out_names.append(name)
            out_avals.append(jax.core.ShapedArray(shape, dtype))
    n_params = len(in_names)
    n_outs = len(out_avals)
    dbg_name = nc.dbg_addr.name if nc.dbg_addr is not None else None
    if dbg_name is not None and dbg_name in in_names:
        in_names.remove(dbg_name)
        n_params -= 1
    in_names_full = (list(in_names)
                     + ([dbg_name] if dbg_name else [])
                     + out_names
                     + ([partition_name] if partition_name else []))
    n_extra = 1 if dbg_name else 0
    donate = tuple(range(n_params + n_extra, n_params + n_extra + n_outs))

    def _body(*args):
        operands = list(args)
        if partition_name is not None:
            operands.append(partition_id_tensor())
        outs = _bass_exec_p.bind(
            *operands,
            out_avals=tuple(out_avals),
            in_names=tuple(in_names_full),
            out_names=tuple(out_names),
            lowering_input_output_aliases=(),
            sim_require_finite=True,
            sim_require_nnan=True,
            nc=nc,
        )
        return tuple(outs)

    devices = jax.devices()[:N_CORES]
    mesh = Mesh(np.asarray(devices), ("core",))
    n_all = n_params + n_extra + n_outs
    sharded = jax.jit(
        shard_map(_body, mesh=mesh,
                  in_specs=(PartitionSpec("core"),) * n_all,
                  out_specs=(PartitionSpec("core"),) * n_outs,
                  check_rep=False),
        donate_argnums=donate, keep_unused=True)
    pool = ThreadPoolExecutor(N_CORES)

    def run(in_map):
        args = [in_map[nm] for nm in in_names]
        if dbg_name:
            args.append(np.zeros((N_CORES, 2), np.uint32))
        zeros = [np.zeros((N_CORES * a.shape[0], *a.shape[1:]), a.dtype)
                 for a in out_avals]
        outs = sharded(*args, *zeros)   # async dispatch

        def fetch():
            res = {}
            for i, nm in enumerate(out_names):
                parts = list(pool.map(
                    lambda s: np.asarray(s.data), outs[i].addressable_shards))
                res[nm] = np.stack(parts).reshape(
                    N_CORES, *out_avals[i].shape)
            return res
        return fetch

    _PROG_CACHE[key] = run
    return run


def _host_prep_global(feats, lengths, transitions):
    """Vectorized build of the globally-concatenated device inputs.

    Global layout = per-core arrays concatenated on axis 0 (core-major);
    feats holds the n_chunk*8 sequences of this chunk, core-major.
    Returns (in_map, coff): the host adds (L-1)*coff per sequence, where
    coff = c_norm (the exp-domain normalizer applied on device) minus the
    quantization-noise bias ln(sinh(h)/h) accrued per live step.
    """
    Bc = feats.shape[0]
    n = Bc // N_CORES
    L = lengths.astype(np.int64)                  # (Bc,)
    M = np.where(L <= 383, 127, 255)              # (B,) split point
    t64 = transitions.astype(np.float64)
    E = np.exp(t64)
    Wb = np.zeros((128, 128), np.float32)
    Wb[0:64, 0:64] = E
    Wb[64:128, 64:128] = E.T
    w_blocks_g = np.tile(Wb.astype(BF16), (N_CORES, 1))

    # quantize feats to q in [-QLEVELS, QLEVELS]; scale ships as an AP.
    # absmax from a subsample (the explicit clip bounds the rare overshoot)
    # and a cache-blocked quantize+pack: one streaming read of feats, one
    # int8 write, instead of five full-memory sweeps.  The exp-domain
    # normalizer c must track the per-step log growth of the partition or
    # the state over/underflows f32 across 256 steps; the growth is close
    # to E[logsumexp of T emissions], estimated here by bootstrap groups
    # of T drawn from the same subsample (closed forms like
    # log T + m + var/2 overshoot badly once the spread is large).
    sub = np.ascontiguousarray(feats.reshape(-1)[::499]).astype(np.float64)
    absmax = float(np.abs(sub).max()) * 1.04
    grp = sub[:(sub.size // T) * T].reshape(-1, T)
    gmx = grp.max(axis=1)
    c_norm = float(
        (np.log(np.exp(grp - gmx[:, None]).sum(axis=1)) + gmx).mean())
    qs = max(absmax, 1e-30) / QLEVELS
    inv = np.float32(1.0 / qs)
    hc = T // 2 if PACK4 else T
    fq = np.empty((feats.shape[0], S, hc), np.int8)
    blk = 16
    tmp = np.empty((blk, S, T), np.float32)
    for b0 in range(0, feats.shape[0], blk):
        nb = min(blk, feats.shape[0] - b0)
        t = tmp[:nb]
        np.multiply(feats[b0:b0 + nb], inv, out=t)
        np.rint(t, out=t)
        np.clip(t, -QLEVELS, QLEVELS, out=t)
        q = t.astype(np.int8)
        if PACK4:
            # byte b holds tag pair (2b, 2b+1): lo nibble even, hi odd
            fq[b0:b0 + nb] = (q[:, :, 1::2] << 4) | (q[:, :, 0::2] & 0x0F)
        else:
            fq[b0:b0 + nb] = q
    h = 0.5 * qs
    delta = float(np.log(np.sinh(h) / h)) if h > 1e-12 else 0.0

    # schedule columns: step i (1..256) consumes column j = i-1.
    # fwd chain of seq v: col j holds feats[s] with s = j - (255-M), live
    # for 1 <= s <= M (tags 0:64).  bwd chain: s = 256 + M - j, live for
    # M+1 <= s <= L-1 (tags 64:128).  Dead slots are 0 -> exp(-c).
    # The shift (255-M) is only ever 0 or 128, so the gathers are two
    # boolean-row slice copies (sequential memcpy beats take_along_axis
    # ~4x); the validity-mask multiply zeroes the dead slots after.
    j = np.arange(NSTEP)[None, :]                 # (1,256)
    sf = j - (255 - M)[:, None]                   # (Bc,256)
    vf = (sf >= 1) & (sf <= M[:, None])
    sb = 256 + M[:, None] - j                     # (Bc,256)
    vb = (sb > M[:, None]) & (sb <= (L - 1)[:, None])
    m255 = M == 255
    m127 = ~m255
    sched = np.empty((Bc, NSTEP, 2 * hc), np.int8)
    gf = sched[:, :, 0:hc]
    gb = sched[:, :, hc:2 * hc]
    fqr = fq[:, ::-1]                             # fqr[v, j] = fq[v, 511-j]
    gf[m255] = fq[m255, 0:NSTEP]
    gf[m127, 128:NSTEP] = fq[m127, 0:128]
    gb[m255] = fqr[m255, 0:NSTEP]
    gb[m127] = fqr[m127, 128:128 + NSTEP]
    gf *= vf[:, :, None]
    gb *= vb[:, :, None]
    # window-major per-core rows: row = w*1024 + v*16 + c_i
    fs_g = (sched.reshape(N_CORES, n, NSTEP // 16, 16, 2 * hc)
                 .transpose(0, 2, 1, 3, 4)
                 .reshape(N_CORES * n * NSTEP, 2 * hc))

    # fwd boots: w0 = exp(feats[:,0] + trans[START]); injected at step 2
    # (M=255 chains, IA) or step 130 (M=127 chains, IB)
    w0 = np.exp(feats[:, 0, :].astype(np.float64) + t64[START][None, :])
    ia = np.where((M == 255)[:, None], w0, 0.0)
    ib = np.where((M == 127)[:, None], w0, 0.0)
    ia_g = ia.reshape(N_CORES, n, T).transpose(0, 2, 1).reshape(N_CORES * T, n)
    ib_g = ib.reshape(N_CORES, n, T).transpose(0, 2, 1).reshape(N_CORES * T, n)

    # bwd boots: delta at STOP injected at step i0b = 258 + M - L, encoded
    # rank-1 as E[:,STOP] (est row) x one-hot boot-mask row
    est_g = np.tile(E[:, STOP].astype(BF16)[None, :], (N_CORES, 1))
    i0b = 258 + M - L                             # (Bc,) in [1, 129]
    mrow_g = np.zeros((N_CORES, NBOOT_BWD * n), BF16)
    bidx = np.arange(Bc)
    mrow_g[bidx // n, (i0b - 1) * n + (bidx % n)] = 1.0

    # ACT scale/bias table: [qs, -(c+8qs), qs/16, -c] per partition
    qs_g = np.tile(
        np.array([[qs, -(c_norm + 8.0 * qs), qs / 16.0, -c_norm]],
                 np.float32),
        (N_CORES * 128, 1))

    return {
        "feats_sched": fs_g,
        "w_blocks": w_blocks_g,
        "inj_a": ia_g.astype(BF16),
        "inj_b": ib_g.astype(BF16),
        "est": est_g,
        "mrow": mrow_g,
        "qscale": qs_g,
    }, c_norm - delta


def _gold_score(feats, mask, tags, transitions):
    t64 = transitions.astype(np.float64)
    prev = np.concatenate(
        [np.full((B, 1), START, dtype=tags.dtype), tags[:, :-1]], axis=1)
    emit = np.take_along_axis(
        feats, tags[:, :, None].astype(np.int64), axis=2)[:, :, 0]
    tg = emit.astype(np.float64) + t64[prev, tags]
    gold = np.where(mask, tg, 0.0).sum()
    lengths = mask.sum(axis=1).astype(np.int64)
    end_ids = np.take_along_axis(tags, (lengths - 1)[:, None].astype(tags.dtype),
                                 axis=1)[:, 0]
    return gold + t64[end_ids, STOP].sum()


def kernel(feats, mask, tags, transitions, _trace=False):
    feats = np.asarray(feats, dtype=np.float32)
    mask = np.asarray(mask)
    tags = np.asarray(tags)
    transitions = np.asarray(transitions, dtype=np.float32)
    lengths = mask.astype(np.int64).sum(axis=1)

    # chunk c takes slots [cW, cW+W) of every core (core-major order);
    # with one chunk the identity slice skips a 64MB fancy-index copy
    W = SEQ_PER_CORE // N_CHUNKS
    n = W
    if N_CHUNKS == 1:
        chunk_idx = [slice(None)]
    else:
        chunk_idx = [
            (np.arange(N_CORES)[:, None] * SEQ_PER_CORE
             + c * W + np.arange(W)[None, :]).ravel()
            for c in range(N_CHUNKS)]

    if _trace:
        from concourse.bass_utils import run_bass_kernel_spmd
        nc = _get_program(n)
        forward = 0.0
        for c in range(N_CHUNKS):
            bi = chunk_idx[c]
            glob_in, coff = _host_prep_global(
                feats[bi], lengths[bi], transitions)
            in_maps = []
            for k in range(N_CORES):
                m = {}
                for nm, g in glob_in.items():
                    rows = g.shape[0] // N_CORES
                    m[nm] = g[k * rows:(k + 1) * rows]
                in_maps.append(m)
            res = run_bass_kernel_spmd(
                nc, in_maps, core_ids=list(range(N_CORES)), trace=True)
            _PROG_CACHE["last_result"] = res
            svec = np.concatenate(
                [res.results[k]["out_s"][0].astype(np.float64)
                 for k in range(N_CORES)])
            l1 = lengths[bi].astype(np.float64) - 1.0
            forward += (np.log(svec) + l1 * coff).sum()
        gold = _gold_score(feats, mask, tags, transitions)
        return np.float32(forward - gold)

    runner = _get_runner(n)
    fetches = []
    coffs = []
    glob_ins = []
    for c in range(N_CHUNKS):
        bi = chunk_idx[c]
        glob_in, coff = _host_prep_global(
            feats[bi], lengths[bi], transitions)
        fetches.append(runner(glob_in))          # async dispatch
        coffs.append(coff)
        glob_ins.append(glob_in)
    gold = _gold_score(feats, mask, tags, transitions)
    forward = 0.0
    for c in range(N_CHUNKS):
        l1 = lengths[chunk_idx[c]].astype(np.float64) - 1.0
        svec = fetches[c]()["out_s"].reshape(-1).astype(np.float64)
        logs = np.log(np.where(svec > 0.0, svec, np.nan))
        # out-of-distribution inputs can drift the exp-domain state past
        # f32 range; recenter c by the realized per-step drift and retry
        # (only the 4KB scale/bias table changes, not the 8MB schedule)
        for _ in range(3):
            fin = np.isfinite(logs)
            if fin.all():
                break
            if fin.any():
                adj = float(np.mean(logs[fin] / np.maximum(l1[fin], 1.0)))
            else:
                adj = 0.3 if not np.isfinite(svec).all() else -0.3
            coffs[c] += adj
            qt = glob_ins[c]["qscale"].copy()
            qt[:, 1] -= adj
            qt[:, 3] -= adj
            glob_ins[c]["qscale"] = qt
            svec = runner(glob_ins[c])()["out_s"].reshape(-1).astype(
                np.float64)
            logs = np.log(np.where(svec > 0.0, svec, np.nan))
        forward += (logs + l1 * coffs[c]).sum()
    return np.float32(forward - gold)
